# revision 25
# baseline (speedup 1.0000x reference)
"""Trainium2 Bass kernel for nn_AttentionBlock (B=4, S=2048, D=1024, H=16, Dh=64).

Sharding: 8 cores = 4 batches x 2 head-groups (8 heads each). Every core runs
the same Bass program on different input slices. The output projection is
row-sharded over head-groups, so the host sums the two partial outputs per
batch (the "all-reduce" of the sharding hint, done on host since we return
full outputs anyway).

Per-core pipeline (all matmuls bf16):
  A) QKV projection: lhsT = X^T chunks [128,128], rhs = Wqkv [128,1536 cols]
     -> psum [128(S-tile), 512] per q/k/v. The psum is copied to SBUF bf16
     (DVE), RoPE applied with bf16 2x-mode DVE ops (4 independent temps),
     then PE-transposed per head into qT/kT [Dh, S] packs. V goes to SBUF
     augmented with a ones column (V_aug [Sk,65]).
  B) Attention, two heads (one partition-half each) interleaved per k-tile:
     sT[Sk-tile 128, Sq 512] = kT_tile.T @ qT_group (alternating PE row
     groups so LDWEIGHTS hides). exp on ScalarE (PSUM->SBUF bf16). The
     causal mask is a [128,128] upper-tri multiply on DVE applied to the
     exp output of diagonal tiles only. AV: x_aug^T[65, Sq] += V_aug.T @ pT
     with row 64 accumulating the softmax denominator Z for free.
     Normalization: the two Z rows are copied to a [2,512] strip (partitions
     0/1), reciprocal to bf16 on DVE (issued early, per pair), then ONE
     K=2 bf16 matmul with a [2,128] selector against the rz strip broadcasts
     1/Z to all 128 partitions, and a single DVE multiply writes xT.
  C) Output projection: out[Sq,512] += xT_pair.T @ WoutPair, DMA to HBM.
     For the last q-group the accumulation is pair-outer so it pipelines
     with the per-pair normalization, shortening the kernel tail.

Issue order software-pipelines groups: QKV chunks of group g+1 and output
chunks of group g-1 are interleaved between attention head-pairs of group g
so the PE always has filler work while ScalarE runs exp.
"""

import sys

for _p in ("/opt/pypackages", "/opt/trn_rl_repo"):
    if _p not in sys.path:
        sys.path.insert(0, _p)

import numpy as np
import ml_dtypes

BF16 = ml_dtypes.bfloat16

B, S, D, H, Dh = 4, 2048, 1024, 16, 64
HL = H // 2          # heads per core
NCORES = 8
ST = S // 128        # 16 S-tiles of 128
NG = S // 512        # 4 q-groups of 512
MAX_WAVELENGTH = 10000.0

_CACHE = {}


def _build_bass():
    import concourse.bass as bass
    import concourse.mybir as mybir
    from concourse import bacc
    from concourse.tile import TileContext
    from contextlib import ExitStack

    f32 = mybir.dt.float32
    bf16 = mybir.dt.bfloat16
    AT = mybir.ActivationFunctionType
    OP = mybir.AluOpType

    nc = bacc.Bacc("TRN2", target_bir_lowering=False)

    # All inputs arrive pre-arranged on the host into the exact SBUF layouts
    # (partition-major, contiguous per partition) so every input DMA moves
    # large contiguous runs instead of 1KB gather descriptors.
    xt_d = nc.dram_tensor("xt", [128, ST, 8, 128], bf16, kind="ExternalInput")
    wqkv_d = nc.dram_tensor("wqkv", [128, 3, 8, 512], bf16, kind="ExternalInput")
    wout_d = nc.dram_tensor("woutp", [128, 4, D], bf16, kind="ExternalInput")
    cos_d = nc.dram_tensor("cost", [128, ST, Dh // 2], bf16, kind="ExternalInput")
    sin_d = nc.dram_tensor("sint", [128, ST, Dh // 2], bf16, kind="ExternalInput")
    identb_d = nc.dram_tensor("identb", [128, 128], bf16, kind="ExternalInput")
    keept_d = nc.dram_tensor("keept", [128, 128], bf16, kind="ExternalInput")
    out_d = nc.dram_tensor("out", [S, D], f32, kind="ExternalOutput")

    with TileContext(nc) as tc, ExitStack() as ctx:
        consts = ctx.enter_context(tc.tile_pool(name="consts", bufs=1))
        persist = ctx.enter_context(tc.tile_pool(name="persist", bufs=1))

        # DMA order matters: what a_proj(0) needs comes first so the PE can
        # start while the rest of the inputs stream in. Two queues (sync for
        # weights/consts, gpsimd for activations) run in parallel.
        wq_sb = consts.tile([128, 3, 8, 512], bf16, tag="wqkv")
        xt_full = consts.tile([128, ST, 8, 128], bf16, tag="xtf")
        identb_sb = consts.tile([128, 128], bf16, tag="identb")
        cos_sb = consts.tile([128, ST, 32], bf16, tag="cos")
        sin_sb = consts.tile([128, ST, 32], bf16, tag="sin")
        nc.gpsimd.dma_start(xt_full[:, 0, :, :], xt_d[:, 0, :, :])
        nc.sync.dma_start(wq_sb[:, 0, 0:4, :], wqkv_d[:, 0, 0:4, :])
        nc.sync.dma_start(wq_sb[:, 0, 4:8, :], wqkv_d[:, 0, 4:8, :])
        nc.gpsimd.dma_start(cos_sb[:, 0:4, :], cos_d[:, 0:4, :])
        nc.gpsimd.dma_start(sin_sb[:, 0:4, :], sin_d[:, 0:4, :])
        nc.sync.dma_start(wq_sb[:, 1, 0:4, :], wqkv_d[:, 1, 0:4, :])
        nc.sync.dma_start(wq_sb[:, 1, 4:8, :], wqkv_d[:, 1, 4:8, :])
        nc.gpsimd.dma_start(xt_full[:, 1, :, :], xt_d[:, 1, :, :])
        nc.sync.dma_start(wq_sb[:, 2, :, :], wqkv_d[:, 2, :, :])
        nc.gpsimd.dma_start(cos_sb[:, 4:ST, :], cos_d[:, 4:ST, :])
        nc.gpsimd.dma_start(sin_sb[:, 4:ST, :], sin_d[:, 4:ST, :])
        nc.sync.dma_start(identb_sb, identb_d[:, :])
        for sc in range(2, 6):
            nc.gpsimd.dma_start(xt_full[:, sc, :, :], xt_d[:, sc, :, :])
        keept_sb = consts.tile([128, 128], bf16, tag="keept")
        nc.sync.dma_start(keept_sb, keept_d[:, :])
        for sc in range(6, ST):
            nc.gpsimd.dma_start(xt_full[:, sc, :, :], xt_d[:, sc, :, :])
        wout_sb = consts.tile([128, 4, 1024], bf16, tag="wout")
        nc.sync.dma_start(wout_sb, wout_d[:, :, :])

        qT = persist.tile([128, 4, S], bf16, tag="qT")
        kT = persist.tile([128, 4, S], bf16, tag="kT")
        xT = persist.tile([128, 4, S], bf16, tag="xT")
        vaug = persist.tile([128, HL, ST, Dh + 1], bf16, tag="vaug")
        nc.scalar.activation(
            vaug[:, :, :, Dh : Dh + 1],
            identb_sb[:, 0:1, None].to_broadcast((128, HL, ST, 1)),
            AT.Identity, bias=1.0, scale=0.0,
        )

        rw_pool = ctx.enter_context(tc.tile_pool(name="ropew", bufs=3))
        pt_pool = ctx.enter_context(tc.tile_pool(name="ptp", bufs=6))
        nrm_pool = ctx.enter_context(tc.tile_pool(name="nrm", bufs=2))
        xus_pool = ctx.enter_context(tc.tile_pool(name="xus", bufs=5))
        out_pool = ctx.enter_context(tc.tile_pool(name="outp", bufs=4))
        psQ = ctx.enter_context(tc.tile_pool(name="psQ", bufs=2, space="PSUM"))
        psS = ctx.enter_context(tc.tile_pool(name="psS", bufs=2, space="PSUM"))
        psX = ctx.enter_context(tc.tile_pool(name="psX", bufs=2, space="PSUM"))

        def a_proj_qk(si, qkv, rots):
            # q or k projection chunk + RoPE (DVE)
            cos_b = cos_sb[:, si, None, :].to_broadcast((128, HL, 32))
            sin_b = sin_sb[:, si, None, :].to_broadcast((128, HL, 32))
            ps = psQ.tile([128, 512], f32, tag="pqkv", name=f"ps{qkv}")
            for c in range(8):
                nc.tensor.matmul(
                    ps,
                    xt_full[:, si, c, :],
                    wq_sb[:, qkv, c, :],
                    start=(c == 0), stop=(c == 7),
                )
            xsb = rw_pool.tile([128, HL, Dh], bf16, tag="xsb")
            if si < 4:
                # prologue: ScalarE is idle until the first exp
                nc.scalar.copy(xsb, ps.rearrange("p (h d) -> p h d", h=HL))
            else:
                nc.vector.tensor_copy(
                    xsb, ps.rearrange("p (h d) -> p h d", h=HL)
                )
            x1, x2 = xsb[:, :, 0:32], xsb[:, :, 32:64]
            rot = rw_pool.tile([128, HL, Dh], bf16, tag="rot")
            t1 = rw_pool.tile([128, HL, 32], bf16, tag="t1")
            t2 = rw_pool.tile([128, HL, 32], bf16, tag="t2")
            t3 = rw_pool.tile([128, HL, 32], bf16, tag="t3")
            t4 = rw_pool.tile([128, HL, 32], bf16, tag="t4")
            nc.vector.tensor_tensor(t1, x1, cos_b, OP.mult)
            nc.vector.tensor_tensor(t2, x2, sin_b, OP.mult)
            nc.vector.tensor_tensor(t3, x1, sin_b, OP.mult)
            nc.vector.tensor_tensor(t4, x2, cos_b, OP.mult)
            nc.vector.tensor_tensor(rot[:, :, 0:32], t1, t2, OP.subtract)
            nc.vector.tensor_tensor(rot[:, :, 32:64], t3, t4, OP.add)
            rots.append(rot)

        def a_proj_v(si):
            ps_v = psQ.tile([128, 512], f32, tag="pqkv", name="ps_v")
            for c in range(8):
                nc.tensor.matmul(
                    ps_v,
                    xt_full[:, si, c, :],
                    wq_sb[:, 2, c, :],
                    start=(c == 0), stop=(c == 7),
                )
            if si < 4:
                nc.scalar.copy(
                    vaug[:, :, si, 0:Dh],
                    ps_v.rearrange("p (h d) -> p h d", h=HL),
                )
            else:
                nc.vector.tensor_copy(
                    vaug[:, :, si, 0:Dh],
                    ps_v.rearrange("p (h d) -> p h d", h=HL),
                )

        def a_proj(si):
            rots = []
            a_proj_qk(si, 0, rots)
            a_proj_qk(si, 1, rots)
            a_proj_v(si)
            return rots

        def a_tail_one(si, rots, idx):
            dstT = (qT, kT)[idx]
            rotf = rots[idx].rearrange("p h d -> p (h d)")
            ps_t = psQ.tile([128, 512], f32, tag="pqkv", name="ps_t")
            for jj in range(4):
                nc.tensor.matmul(
                    ps_t[:, jj * 128 : (jj + 1) * 128],
                    rotf[:, jj * 128 : (jj + 1) * 128],
                    identb_sb, start=True, stop=True,
                )
            if si < 4:
                nc.scalar.copy(
                    dstT[:, :, si * 128 : (si + 1) * 128],
                    ps_t.rearrange("p (j s) -> p j s", j=4),
                )
            else:
                nc.vector.tensor_copy(
                    dstT[:, :, si * 128 : (si + 1) * 128],
                    ps_t.rearrange("p (j s) -> p j s", j=4),
                )

        def a_tail(si, rots):
            a_tail_one(si, rots, 0)
            a_tail_one(si, rots, 1)

        def b_pair(g, hh, zp, fillers=None):
            nj = 4 * (g + 1)
            px = [
                psX.tile([Dh + 1, 512], f32, tag="psx", name=f"px{hp}")
                for hp in range(2)
            ]

            def flush_av(ss):
                for j, c0, pt in ss:
                    for hp in range(2):
                        nc.tensor.matmul(
                            px[hp][:, c0:512],
                            vaug[:, 2 * hh + hp, j, :], pt[:, hp, c0:512],
                            start=(j == 0), stop=(j == nj - 1),
                        )

            # software pipeline: the AV matmuls for iteration jj are issued
            # AFTER iteration jj+2's score matmuls, so their exp dependency
            # has a full iteration of PE work to resolve (no PE stall).
            prev = None
            for jj in range(0, nj, 2):
                ss = []
                for j in (jj, jj + 1):
                    diag = j >= 4 * g
                    c0 = 128 * (j - 4 * g) if diag else 0
                    ps_s = psS.tile([128, 2, 512], f32, tag="pss")
                    for hp in range(2):
                        nc.tensor.matmul(
                            ps_s[:, hp, c0:512],
                            kT[64 * hp : 64 * hp + 64, hh,
                               j * 128 : (j + 1) * 128],
                            qT[64 * hp : 64 * hp + 64, hh,
                               g * 512 + c0 : (g + 1) * 512],
                            start=True, stop=True,
                        )
                    pt = pt_pool.tile([128, 2, 512], bf16, tag="pt")
                    nc.scalar.activation(pt[:, :, c0:512], ps_s[:, :, c0:512],
                                         AT.Exp)
                    if diag:
                        for hp in range(2):
                            nc.vector.tensor_tensor(
                                pt[:, hp, c0 : c0 + 128],
                                pt[:, hp, c0 : c0 + 128],
                                keept_sb, OP.mult,
                            )
                    ss.append((j, c0, pt))
                for _ in range(2):
                    if fillers:
                        # independent filler work (projections for the next
                        # group, output chunks, pair norms) issued between
                        # the exp and its consuming AV matmuls: the PE
                        # streams the filler while ScalarE catches up,
                        # instead of stalling
                        fillers.popleft()()
                flush_av(ss)
            # stash the two Z rows (bf16) into the pair's [2,512] strip at
            # partitions 0/1. Row 0 is a direct partition-0 DVE copy; row 1
            # bounces through a partition-0 tile + gpsimd DMA (engine APs
            # must start at a 32-aligned partition).
            last = g == NG - 1 and hh == 3
            eng = nc.scalar if last else nc.vector
            cp = eng.copy if last else eng.tensor_copy
            cp(zp[0:1, :], px[0][Dh : Dh + 1, :])
            ztb = nrm_pool.tile([1, 512], bf16, tag="ztb")
            cp(ztb, px[1][Dh : Dh + 1, :])
            nc.gpsimd.dma_start(zp[1:2, :], ztb)
            xus = xus_pool.tile([128, 512], bf16, tag="xus", name=f"xus{hh}")
            nc.vector.tensor_copy(xus[0:64, :], px[0][0:Dh, :])
            nc.vector.tensor_copy(xus[64:128, :], px[1][0:Dh, :])
            return xus

        def b_norm_zc(zp, pool, tag):
            # Z rows -> columns with 4 tiny K=2 bf16 transposes, reciprocal
            # on DVE in column form (8 free elems)
            zcv = pool.tile([128, 4, 2], f32, tag=tag, name="zcv")
            for m in range(4):
                nc.tensor.matmul(
                    zcv[:, m, :], zp[:, m * 128 : (m + 1) * 128],
                    identb_sb[0:2, 0:2], start=True, stop=True,
                )
            rcol = nrm_pool.tile([128, 4, 2], f32, tag="rcol")
            nc.vector.reciprocal(rcol, zcv)
            rcolb = nrm_pool.tile([128, 4, 2], bf16, tag="rcolb")
            nc.vector.tensor_copy(rcolb, rcol)
            return rcolb

        def b_norm_bc(g, hh, rcolb, xus_l, pool, tag):
            # 8 bf16 broadcast matmuls rebuild 1/Z across all 128 partitions
            bc = pool.tile([128, 512], f32, tag=tag, name="bc")
            for m in range(4):
                for hp in range(2):
                    nc.tensor.matmul(
                        bc[64 * hp : 64 * hp + 64, m * 128 : (m + 1) * 128],
                        rcolb[:, m, hp : hp + 1].to_broadcast((128, 64)),
                        identb_sb, start=True, stop=True,
                    )
            nc.vector.tensor_tensor(
                xT[:, hh, g * 512 : (g + 1) * 512], xus_l[hh], bc, OP.mult
            )

        def b_norm_pair(g, hh, zp, xus_l):
            rcolb = b_norm_zc(zp, psQ, "pqkv")
            b_norm_bc(g, hh, rcolb, xus_l, psQ, "pqkv")

        def c_chunk(m, scalar_copy=False):
            for half in range(2):
                ps_o = psQ.tile([128, 512], f32, tag="pqkv", name="ps_o")
                for p in range(4):
                    nc.tensor.matmul(
                        ps_o,
                        xT[:, p, m * 128 : (m + 1) * 128],
                        wout_sb[:, p, half * 512 : (half + 1) * 512],
                        start=(p == 0),
                        stop=(p == 3),
                    )
                ob = out_pool.tile([128, 512], f32, tag="ob")
                if scalar_copy:
                    # epilogue: all exp work is done, ScalarE is idle
                    nc.scalar.copy(ob, ps_o)
                else:
                    nc.vector.tensor_copy(ob, ps_o)
                nc.sync.dma_start(
                    out_d[m * 128 : (m + 1) * 128,
                          half * 512 : (half + 1) * 512],
                    ob,
                )

        from collections import deque

        fillers = deque()

        pend = None
        for si in range(4):
            r = a_proj(si)
            if pend is not None:
                a_tail(*pend)
            pend = (si, r)
        a_tail(*pend)
        for g in range(NG):
            zps = [
                nrm_pool.tile([2, 512], bf16, tag=f"zp{hh}", name=f"zp{hh}")
                for hh in range(4)
            ]
            xus_l = []
            pend = None
            for hh in range(4):
                # queue this iteration's independent work as fillers; b_pair
                # pops one between each pair of attention k-tiles
                if hh >= 1:
                    rc_h = []
                    fillers.append(
                        lambda p=hh - 1, rc=rc_h: rc.append(
                            b_norm_zc(zps[p], psQ, "pqkv"))
                    )
                    fillers.append(
                        lambda g=g, p=hh - 1, rc=rc_h: b_norm_bc(
                            g, p, rc[0], xus_l, psQ, "pqkv")
                    )
                if pend is not None:
                    psi, prots = pend
                    fillers.append(
                        lambda si=psi, r=prots: a_tail_one(si, r, 0))
                    fillers.append(
                        lambda si=psi, r=prots: a_tail_one(si, r, 1))
                    pend = None
                if g < NG - 1:
                    si = 4 * (g + 1) + hh
                    rots = []
                    fillers.append(
                        lambda si=si, r=rots: a_proj_qk(si, 0, r))
                    fillers.append(
                        lambda si=si, r=rots: a_proj_qk(si, 1, r))
                    fillers.append(lambda si=si: a_proj_v(si))
                    pend = (si, rots)
                if g >= 1:
                    m = 4 * (g - 1) + hh
                    fillers.append(lambda m=m: c_chunk(m))
                xus_l.append(b_pair(g, hh, zps[hh], fillers))
                # drain this iteration's leftovers before the next pair
                while fillers:
                    fillers.popleft()()
            if pend is not None:
                a_tail(*pend)
            if g < NG - 1:
                b_norm_pair(g, 3, zps[3], xus_l)
            else:
                # kernel tail: the last pair's normalization (psS psums; the
                # score psums are drained) is interleaved with the first
                # output chunk's p=0..2 partial accumulation so the PE never
                # idles (and never drops out of its fast p-state).
                rcolb3 = b_norm_zc(zps[3], psS, "pss")
                o12 = [
                    psQ.tile([128, 512], f32, tag="pqkv", name=f"o12h{h}")
                    for h in range(2)
                ]
                for half in range(2):
                    for p in range(3):
                        nc.tensor.matmul(
                            o12[half],
                            xT[:, p, 12 * 128 : 13 * 128],
                            wout_sb[:, p, half * 512 : (half + 1) * 512],
                            start=(p == 0), stop=False,
                        )
                b_norm_bc(g, 3, rcolb3, xus_l, psS, "pss")
                for half in range(2):
                    nc.tensor.matmul(
                        o12[half],
                        xT[:, 3, 12 * 128 : 13 * 128],
                        wout_sb[:, 3, half * 512 : (half + 1) * 512],
                        start=False, stop=True,
                    )
                    ob = out_pool.tile([128, 512], f32, tag="ob")
                    nc.scalar.copy(ob, o12[half])
                    nc.sync.dma_start(
                        out_d[12 * 128 : 13 * 128,
                              half * 512 : (half + 1) * 512],
                        ob,
                    )
        for m in range(13, 16):
            c_chunk(m, scalar_copy=True)

    nc.compile()
    return nc


def _numpy_fallback(x, w_q, w_k, w_v, w_out, seg, mask):
    """Exact numpy replica of the reference for non-causal masks."""
    frac = (2.0 * np.arange(Dh // 2, dtype=np.float32)) / Dh
    ts = (MAX_WAVELENGTH ** frac).astype(np.float32)

    def rope(t, pos):
        sinu = pos.astype(np.float32)[:, :, None] / ts  # [B,S,32]
        sn, cs = np.sin(sinu), np.cos(sinu)
        sn, cs = sn[:, :, None, :], cs[:, :, None, :]
        f, s_ = t[..., :32], t[..., 32:]
        return np.concatenate([f * cs - s_ * sn, s_ * cs + f * sn], -1)

    q = np.einsum("bsd,dhk->bshk", x, w_q)
    k = np.einsum("bsd,dhk->bshk", x, w_k)
    v = np.einsum("bsd,dhk->bshk", x, w_v)
    q, k = rope(q, seg), rope(k, seg)
    q = q / np.sqrt(np.float32(Dh))
    attn = np.einsum("bqhd,bkhd->bhqk", q, k)
    attn = np.where(mask, attn, np.finfo(np.float32).min)
    attn = attn - attn.max(-1, keepdims=True)
    e = np.exp(attn)
    attn = e / e.sum(-1, keepdims=True)
    xo = np.einsum("bhqk,bkhd->bqhd", attn, v)
    return np.einsum("bqhd,hdm->bqm", xo, w_out).astype(np.float32)


def _host_inputs(x, w_q, w_k, w_v, w_out, seg):
    frac = (2.0 * np.arange(Dh // 2, dtype=np.float32)) / Dh
    ts = (MAX_WAVELENGTH ** frac).astype(np.float32)
    identb = np.eye(128, dtype=np.float32).astype(BF16)
    keept = np.triu(np.ones((128, 128), dtype=np.float32)).astype(BF16)

    in_maps = []
    for core in range(NCORES):
        b, g = core // 2, core % 2
        hs = slice(g * HL, (g + 1) * HL)
        wq_s = (w_q[:, hs, :] / np.float32(np.sqrt(Dh))).reshape(D, HL * Dh)
        wk_s = w_k[:, hs, :].reshape(D, HL * Dh)
        wv_s = w_v[:, hs, :].reshape(D, HL * Dh)
        wqkv = np.ascontiguousarray(
            np.concatenate([wq_s, wk_s, wv_s], axis=1), dtype=np.float32
        ).astype(BF16)
        woutp = np.stack(
            [
                w_out[g * HL + 2 * p : g * HL + 2 * p + 2].reshape(128, D)
                for p in range(4)
            ]
        ).astype(BF16)
        sinu = seg[b].astype(np.float32)[:, None] / ts  # [S, 32]
        # pre-arrange into partition-major SBUF layouts (contiguous DMAs),
        # si-major so the first S-tile's chunk arrives first
        xt5 = (
            x[b].T.reshape(8, 128, ST, 128).transpose(1, 2, 0, 3)
        )  # [p, si, c, t]
        wq4 = wqkv.astype(np.float32).reshape(8, 128, 3, 512).transpose(
            1, 2, 0, 3
        )  # [p, qkv, c, n]
        wo3 = woutp.transpose(1, 0, 2)  # [p, q, n]
        cos3 = np.cos(sinu).reshape(ST, 128, 32).transpose(1, 0, 2)
        sin3 = np.sin(sinu).reshape(ST, 128, 32).transpose(1, 0, 2)
        in_maps.append(
            {
                "xt": np.ascontiguousarray(xt5).astype(BF16),
                "wqkv": np.ascontiguousarray(wq4).astype(BF16),
                "woutp": np.ascontiguousarray(wo3).astype(BF16),
                "cost": np.ascontiguousarray(cos3).astype(BF16),
                "sint": np.ascontiguousarray(sin3).astype(BF16),
                "identb": identb,
                "keept": keept,
            }
        )
    return in_maps


def _run(in_maps, trace=False):
    from concourse.bass_utils import run_bass_kernel_spmd

    if "nc" not in _CACHE:
        _CACHE["nc"] = _build_bass()
    return run_bass_kernel_spmd(
        _CACHE["nc"], in_maps, core_ids=list(range(NCORES)), trace=trace
    )


def kernel(**inputs):
    x = np.asarray(inputs["inputs"], dtype=np.float32)
    w_q = np.asarray(inputs["w_q"], dtype=np.float32)
    w_k = np.asarray(inputs["w_k"], dtype=np.float32)
    w_v = np.asarray(inputs["w_v"], dtype=np.float32)
    w_out = np.asarray(inputs["w_out"], dtype=np.float32)
    seg = np.asarray(inputs["segment_positions"])
    mask = np.asarray(inputs["mask"])

    causal = np.tril(np.ones((S, S), dtype=bool))
    if not all(np.array_equal(mask[b, 0], causal) for b in range(B)):
        return _numpy_fallback(x, w_q, w_k, w_v, w_out, seg, mask)

    in_maps = _host_inputs(x, w_q, w_k, w_v, w_out, seg)
    res = _run(in_maps)
    outs = [r_["out"] for r_ in res.results]
    result = np.empty((B, S, D), dtype=np.float32)
    for b in range(B):
        result[b] = outs[2 * b] + outs[2 * b + 1]
    return result


# revision 26
# speedup vs baseline: 1.0210x; 1.0210x over previous
"""Trainium2 Bass kernel for nn_AttentionBlock (B=4, S=2048, D=1024, H=16, Dh=64).

Sharding: 8 cores = 4 batches x 2 head-groups (8 heads each). Every core runs
the same Bass program on different input slices. The output projection is
row-sharded over head-groups, so the host sums the two partial outputs per
batch (the "all-reduce" of the sharding hint, done on host since we return
full outputs anyway).

Per-core pipeline (all matmuls bf16):
  A) QKV projection: lhsT = X^T chunks [128,128], rhs = Wqkv [128,1536 cols]
     -> psum [128(S-tile), 512] per q/k/v. The psum is copied to SBUF bf16
     (DVE), RoPE applied with bf16 2x-mode DVE ops (4 independent temps),
     then PE-transposed per head into qT/kT [Dh, S] packs. V goes to SBUF
     augmented with a ones column (V_aug [Sk,65]).
  B) Attention, two heads (one partition-half each) interleaved per k-tile:
     sT[Sk-tile 128, Sq 512] = kT_tile.T @ qT_group (alternating PE row
     groups so LDWEIGHTS hides). exp on ScalarE (PSUM->SBUF bf16). The
     causal mask is a [128,128] upper-tri multiply on DVE applied to the
     exp output of diagonal tiles only. AV: x_aug^T[65, Sq] += V_aug.T @ pT
     with row 64 accumulating the softmax denominator Z for free.
     Normalization: the two Z rows are copied to a [2,512] strip (partitions
     0/1), reciprocal to bf16 on DVE (issued early, per pair), then ONE
     K=2 bf16 matmul with a [2,128] selector against the rz strip broadcasts
     1/Z to all 128 partitions, and a single DVE multiply writes xT.
  C) Output projection: out[Sq,512] += xT_pair.T @ WoutPair, DMA to HBM.
     For the last q-group the accumulation is pair-outer so it pipelines
     with the per-pair normalization, shortening the kernel tail.

Issue order software-pipelines groups: QKV chunks of group g+1 and output
chunks of group g-1 are interleaved between attention head-pairs of group g
so the PE always has filler work while ScalarE runs exp.
"""

import sys

for _p in ("/opt/pypackages", "/opt/trn_rl_repo"):
    if _p not in sys.path:
        sys.path.insert(0, _p)

import numpy as np
import ml_dtypes

BF16 = ml_dtypes.bfloat16

B, S, D, H, Dh = 4, 2048, 1024, 16, 64
HL = H // 2          # heads per core
NCORES = 8
ST = S // 128        # 16 S-tiles of 128
NG = S // 512        # 4 q-groups of 512
MAX_WAVELENGTH = 10000.0

_CACHE = {}


def _build_bass():
    import concourse.bass as bass
    import concourse.mybir as mybir
    from concourse import bacc
    from concourse.tile import TileContext
    from contextlib import ExitStack

    f32 = mybir.dt.float32
    bf16 = mybir.dt.bfloat16
    AT = mybir.ActivationFunctionType
    OP = mybir.AluOpType

    nc = bacc.Bacc("TRN2", target_bir_lowering=False)

    # All inputs arrive pre-arranged on the host into the exact SBUF layouts
    # (partition-major, contiguous per partition) so every input DMA moves
    # large contiguous runs instead of 1KB gather descriptors.
    xt_d = nc.dram_tensor("xt", [128, ST, 8, 128], bf16, kind="ExternalInput")
    wqkv_d = nc.dram_tensor("wqkv", [128, 3, 8, 512], bf16, kind="ExternalInput")
    wout_d = nc.dram_tensor("woutp", [128, 4, D], bf16, kind="ExternalInput")
    cos_d = nc.dram_tensor("cost", [128, ST, Dh // 2], bf16, kind="ExternalInput")
    sin_d = nc.dram_tensor("sint", [128, ST, Dh // 2], bf16, kind="ExternalInput")
    identb_d = nc.dram_tensor("identb", [128, 128], bf16, kind="ExternalInput")
    keept_d = nc.dram_tensor("keept", [128, 128], bf16, kind="ExternalInput")
    out_d = nc.dram_tensor("out", [S, D], f32, kind="ExternalOutput")

    with TileContext(nc) as tc, ExitStack() as ctx:
        consts = ctx.enter_context(tc.tile_pool(name="consts", bufs=1))
        persist = ctx.enter_context(tc.tile_pool(name="persist", bufs=1))

        # DMA order matters: what a_proj(0) needs comes first so the PE can
        # start while the rest of the inputs stream in. Two queues (sync for
        # weights/consts, gpsimd for activations) run in parallel.
        wq_sb = consts.tile([128, 3, 8, 512], bf16, tag="wqkv")
        xt_full = consts.tile([128, ST, 8, 128], bf16, tag="xtf")
        identb_sb = consts.tile([128, 128], bf16, tag="identb")
        cos_sb = consts.tile([128, ST, 32], bf16, tag="cos")
        sin_sb = consts.tile([128, ST, 32], bf16, tag="sin")
        nc.gpsimd.dma_start(xt_full[:, 0, :, :], xt_d[:, 0, :, :])
        nc.sync.dma_start(wq_sb[:, 0, 0:4, :], wqkv_d[:, 0, 0:4, :])
        nc.sync.dma_start(wq_sb[:, 0, 4:8, :], wqkv_d[:, 0, 4:8, :])
        nc.gpsimd.dma_start(cos_sb[:, 0:4, :], cos_d[:, 0:4, :])
        nc.gpsimd.dma_start(sin_sb[:, 0:4, :], sin_d[:, 0:4, :])
        nc.sync.dma_start(wq_sb[:, 1, 0:4, :], wqkv_d[:, 1, 0:4, :])
        nc.sync.dma_start(wq_sb[:, 1, 4:8, :], wqkv_d[:, 1, 4:8, :])
        nc.gpsimd.dma_start(xt_full[:, 1, :, :], xt_d[:, 1, :, :])
        nc.sync.dma_start(wq_sb[:, 2, :, :], wqkv_d[:, 2, :, :])
        nc.gpsimd.dma_start(cos_sb[:, 4:ST, :], cos_d[:, 4:ST, :])
        nc.gpsimd.dma_start(sin_sb[:, 4:ST, :], sin_d[:, 4:ST, :])
        nc.sync.dma_start(identb_sb, identb_d[:, :])
        for sc in range(2, 6):
            nc.gpsimd.dma_start(xt_full[:, sc, :, :], xt_d[:, sc, :, :])
        keept_sb = consts.tile([128, 128], bf16, tag="keept")
        nc.sync.dma_start(keept_sb, keept_d[:, :])
        for sc in range(6, ST):
            nc.gpsimd.dma_start(xt_full[:, sc, :, :], xt_d[:, sc, :, :])
        wout_sb = consts.tile([128, 4, 1024], bf16, tag="wout")
        nc.sync.dma_start(wout_sb, wout_d[:, :, :])

        qT = persist.tile([128, 4, S], bf16, tag="qT")
        kT = persist.tile([128, 4, S], bf16, tag="kT")
        xT = persist.tile([128, 4, S], bf16, tag="xT")
        vaug = persist.tile([128, HL, ST, Dh + 1], bf16, tag="vaug")
        nc.scalar.activation(
            vaug[:, :, :, Dh : Dh + 1],
            identb_sb[:, 0:1, None].to_broadcast((128, HL, ST, 1)),
            AT.Identity, bias=1.0, scale=0.0,
        )

        rw_pool = ctx.enter_context(tc.tile_pool(name="ropew", bufs=3))
        pt_pool = ctx.enter_context(tc.tile_pool(name="ptp", bufs=6))
        nrm_pool = ctx.enter_context(tc.tile_pool(name="nrm", bufs=2))
        xus_pool = ctx.enter_context(tc.tile_pool(name="xus", bufs=5))
        out_pool = ctx.enter_context(tc.tile_pool(name="outp", bufs=4))
        psQ = ctx.enter_context(tc.tile_pool(name="psQ", bufs=2, space="PSUM"))
        psS = ctx.enter_context(tc.tile_pool(name="psS", bufs=2, space="PSUM"))
        psX = ctx.enter_context(tc.tile_pool(name="psX", bufs=2, space="PSUM"))

        def a_proj_qk(si, qkv, rots):
            # q or k projection chunk + RoPE (DVE)
            cos_b = cos_sb[:, si, None, :].to_broadcast((128, HL, 32))
            sin_b = sin_sb[:, si, None, :].to_broadcast((128, HL, 32))
            ps = psQ.tile([128, 512], f32, tag="pqkv", name=f"ps{qkv}")
            for c in range(8):
                nc.tensor.matmul(
                    ps,
                    xt_full[:, si, c, :],
                    wq_sb[:, qkv, c, :],
                    start=(c == 0), stop=(c == 7),
                )
            xsb = rw_pool.tile([128, HL, Dh], bf16, tag="xsb")
            if si < 4:
                # prologue: ScalarE is idle until the first exp
                nc.scalar.copy(xsb, ps.rearrange("p (h d) -> p h d", h=HL))
            else:
                nc.vector.tensor_copy(
                    xsb, ps.rearrange("p (h d) -> p h d", h=HL)
                )
            x1, x2 = xsb[:, :, 0:32], xsb[:, :, 32:64]
            rot = rw_pool.tile([128, HL, Dh], bf16, tag="rot")
            t1 = rw_pool.tile([128, HL, 32], bf16, tag="t1")
            t2 = rw_pool.tile([128, HL, 32], bf16, tag="t2")
            t3 = rw_pool.tile([128, HL, 32], bf16, tag="t3")
            t4 = rw_pool.tile([128, HL, 32], bf16, tag="t4")
            nc.vector.tensor_tensor(t1, x1, cos_b, OP.mult)
            nc.vector.tensor_tensor(t2, x2, sin_b, OP.mult)
            nc.vector.tensor_tensor(t3, x1, sin_b, OP.mult)
            nc.vector.tensor_tensor(t4, x2, cos_b, OP.mult)
            nc.vector.tensor_tensor(rot[:, :, 0:32], t1, t2, OP.subtract)
            nc.vector.tensor_tensor(rot[:, :, 32:64], t3, t4, OP.add)
            rots.append(rot)

        def a_proj_v(si):
            ps_v = psQ.tile([128, 512], f32, tag="pqkv", name="ps_v")
            for c in range(8):
                nc.tensor.matmul(
                    ps_v,
                    xt_full[:, si, c, :],
                    wq_sb[:, 2, c, :],
                    start=(c == 0), stop=(c == 7),
                )
            if si < 4:
                nc.scalar.copy(
                    vaug[:, :, si, 0:Dh],
                    ps_v.rearrange("p (h d) -> p h d", h=HL),
                )
            else:
                nc.vector.tensor_copy(
                    vaug[:, :, si, 0:Dh],
                    ps_v.rearrange("p (h d) -> p h d", h=HL),
                )

        def a_proj(si):
            rots = []
            a_proj_qk(si, 0, rots)
            a_proj_qk(si, 1, rots)
            a_proj_v(si)
            return rots

        def a_tail_one(si, rots, idx):
            dstT = (qT, kT)[idx]
            rotf = rots[idx].rearrange("p h d -> p (h d)")
            ps_t = psQ.tile([128, 512], f32, tag="pqkv", name="ps_t")
            for jj in range(4):
                nc.tensor.matmul(
                    ps_t[:, jj * 128 : (jj + 1) * 128],
                    rotf[:, jj * 128 : (jj + 1) * 128],
                    identb_sb, start=True, stop=True,
                )
            if si < 4:
                nc.scalar.copy(
                    dstT[:, :, si * 128 : (si + 1) * 128],
                    ps_t.rearrange("p (j s) -> p j s", j=4),
                )
            else:
                nc.vector.tensor_copy(
                    dstT[:, :, si * 128 : (si + 1) * 128],
                    ps_t.rearrange("p (j s) -> p j s", j=4),
                )

        def a_tail(si, rots):
            a_tail_one(si, rots, 0)
            a_tail_one(si, rots, 1)

        def b_pair(g, hh, zp, fillers=None):
            nj = 4 * (g + 1)
            px = [
                psX.tile([Dh + 1, 512], f32, tag="psx", name=f"px{hp}")
                for hp in range(2)
            ]

            def flush_av(ss):
                for j, c0, pt in ss:
                    for hp in range(2):
                        nc.tensor.matmul(
                            px[hp][:, c0:512],
                            vaug[:, 2 * hh + hp, j, :], pt[:, hp, c0:512],
                            start=(j == 0), stop=(j == nj - 1),
                        )

            # software pipeline: the AV matmuls for iteration jj are issued
            # AFTER iteration jj+2's score matmuls, so their exp dependency
            # has a full iteration of PE work to resolve (no PE stall).
            prev = None
            for jj in range(0, nj, 2):
                ss = []
                for j in (jj, jj + 1):
                    diag = j >= 4 * g
                    c0 = 128 * (j - 4 * g) if diag else 0
                    ps_s = psS.tile([128, 2, 512], f32, tag="pss")
                    for hp in range(2):
                        nc.tensor.matmul(
                            ps_s[:, hp, c0:512],
                            kT[64 * hp : 64 * hp + 64, hh,
                               j * 128 : (j + 1) * 128],
                            qT[64 * hp : 64 * hp + 64, hh,
                               g * 512 + c0 : (g + 1) * 512],
                            start=True, stop=True,
                        )
                    pt = pt_pool.tile([128, 2, 512], bf16, tag="pt")
                    nc.scalar.activation(pt[:, :, c0:512], ps_s[:, :, c0:512],
                                         AT.Exp)
                    if diag:
                        for hp in range(2):
                            nc.vector.tensor_tensor(
                                pt[:, hp, c0 : c0 + 128],
                                pt[:, hp, c0 : c0 + 128],
                                keept_sb, OP.mult,
                            )
                    ss.append((j, c0, pt))
                if fillers:
                    # independent filler work (projections for the next
                    # group, output chunks, pair norms) issued between the
                    # exp and its consuming AV matmuls: the PE streams the
                    # filler while ScalarE catches up, instead of stalling
                    fillers.popleft()()
                flush_av(ss)
            # stash the two Z rows (bf16) into the pair's [2,512] strip at
            # partitions 0/1. Row 0 is a direct partition-0 DVE copy; row 1
            # bounces through a partition-0 tile + gpsimd DMA (engine APs
            # must start at a 32-aligned partition).
            last = g == NG - 1 and hh == 3
            eng = nc.scalar if last else nc.vector
            cp = eng.copy if last else eng.tensor_copy
            cp(zp[0:1, :], px[0][Dh : Dh + 1, :])
            ztb = nrm_pool.tile([1, 512], bf16, tag="ztb")
            cp(ztb, px[1][Dh : Dh + 1, :])
            nc.gpsimd.dma_start(zp[1:2, :], ztb)
            xus = xus_pool.tile([128, 512], bf16, tag="xus", name=f"xus{hh}")
            nc.vector.tensor_copy(xus[0:64, :], px[0][0:Dh, :])
            nc.vector.tensor_copy(xus[64:128, :], px[1][0:Dh, :])
            return xus

        def b_norm_zc(zp, pool, tag):
            # Z rows -> columns with 4 tiny K=2 bf16 transposes, reciprocal
            # on DVE in column form (8 free elems)
            zcv = pool.tile([128, 4, 2], f32, tag=tag, name="zcv")
            for m in range(4):
                nc.tensor.matmul(
                    zcv[:, m, :], zp[:, m * 128 : (m + 1) * 128],
                    identb_sb[0:2, 0:2], start=True, stop=True,
                )
            rcol = nrm_pool.tile([128, 4, 2], f32, tag="rcol")
            nc.vector.reciprocal(rcol, zcv)
            rcolb = nrm_pool.tile([128, 4, 2], bf16, tag="rcolb")
            nc.vector.tensor_copy(rcolb, rcol)
            return rcolb

        def b_norm_bc(g, hh, rcolb, xus_l, pool, tag):
            # 8 bf16 broadcast matmuls rebuild 1/Z across all 128 partitions
            bc = pool.tile([128, 512], f32, tag=tag, name="bc")
            for m in range(4):
                for hp in range(2):
                    nc.tensor.matmul(
                        bc[64 * hp : 64 * hp + 64, m * 128 : (m + 1) * 128],
                        rcolb[:, m, hp : hp + 1].to_broadcast((128, 64)),
                        identb_sb, start=True, stop=True,
                    )
            nc.vector.tensor_tensor(
                xT[:, hh, g * 512 : (g + 1) * 512], xus_l[hh], bc, OP.mult
            )

        def b_norm_pair(g, hh, zp, xus_l):
            rcolb = b_norm_zc(zp, psQ, "pqkv")
            b_norm_bc(g, hh, rcolb, xus_l, psQ, "pqkv")

        def c_chunk(m, scalar_copy=False):
            for half in range(2):
                ps_o = psQ.tile([128, 512], f32, tag="pqkv", name="ps_o")
                for p in range(4):
                    nc.tensor.matmul(
                        ps_o,
                        xT[:, p, m * 128 : (m + 1) * 128],
                        wout_sb[:, p, half * 512 : (half + 1) * 512],
                        start=(p == 0),
                        stop=(p == 3),
                    )
                ob = out_pool.tile([128, 512], f32, tag="ob")
                if scalar_copy:
                    # epilogue: all exp work is done, ScalarE is idle
                    nc.scalar.copy(ob, ps_o)
                else:
                    nc.vector.tensor_copy(ob, ps_o)
                nc.sync.dma_start(
                    out_d[m * 128 : (m + 1) * 128,
                          half * 512 : (half + 1) * 512],
                    ob,
                )

        from collections import deque

        fillers = deque()

        pend = None
        for si in range(4):
            r = a_proj(si)
            if pend is not None:
                a_tail(*pend)
            pend = (si, r)
        a_tail(*pend)
        for g in range(NG):
            zps = [
                nrm_pool.tile([2, 512], bf16, tag=f"zp{hh}", name=f"zp{hh}")
                for hh in range(4)
            ]
            xus_l = []
            pend = None
            for hh in range(4):
                # queue this iteration's independent work as fillers; b_pair
                # pops one between each pair of attention k-tiles
                if hh >= 1:
                    rc_h = []
                    fillers.append(
                        lambda p=hh - 1, rc=rc_h: rc.append(
                            b_norm_zc(zps[p], psQ, "pqkv"))
                    )
                    fillers.append(
                        lambda g=g, p=hh - 1, rc=rc_h: b_norm_bc(
                            g, p, rc[0], xus_l, psQ, "pqkv")
                    )
                if pend is not None:
                    psi, prots = pend
                    fillers.append(
                        lambda si=psi, r=prots: a_tail_one(si, r, 0))
                    fillers.append(
                        lambda si=psi, r=prots: a_tail_one(si, r, 1))
                    pend = None
                if g < NG - 1:
                    si = 4 * (g + 1) + hh
                    rots = []
                    fillers.append(
                        lambda si=si, r=rots: a_proj_qk(si, 0, r))
                    fillers.append(
                        lambda si=si, r=rots: a_proj_qk(si, 1, r))
                    fillers.append(lambda si=si: a_proj_v(si))
                    pend = (si, rots)
                if g >= 1:
                    m = 4 * (g - 1) + hh
                    fillers.append(lambda m=m: c_chunk(m))
                xus_l.append(b_pair(g, hh, zps[hh], fillers))
                # drain this iteration's leftovers before the next pair
                while fillers:
                    fillers.popleft()()
            if pend is not None:
                a_tail(*pend)
            if g < NG - 1:
                b_norm_pair(g, 3, zps[3], xus_l)
            else:
                # kernel tail: the last pair's normalization (psS psums; the
                # score psums are drained) is interleaved with the first
                # output chunk's p=0..2 partial accumulation so the PE never
                # idles (and never drops out of its fast p-state).
                rcolb3 = b_norm_zc(zps[3], psS, "pss")
                o12 = [
                    psQ.tile([128, 512], f32, tag="pqkv", name=f"o12h{h}")
                    for h in range(2)
                ]
                for half in range(2):
                    for p in range(3):
                        nc.tensor.matmul(
                            o12[half],
                            xT[:, p, 12 * 128 : 13 * 128],
                            wout_sb[:, p, half * 512 : (half + 1) * 512],
                            start=(p == 0), stop=False,
                        )
                b_norm_bc(g, 3, rcolb3, xus_l, psS, "pss")
                for half in range(2):
                    nc.tensor.matmul(
                        o12[half],
                        xT[:, 3, 12 * 128 : 13 * 128],
                        wout_sb[:, 3, half * 512 : (half + 1) * 512],
                        start=False, stop=True,
                    )
                    ob = out_pool.tile([128, 512], f32, tag="ob")
                    nc.scalar.copy(ob, o12[half])
                    nc.sync.dma_start(
                        out_d[12 * 128 : 13 * 128,
                              half * 512 : (half + 1) * 512],
                        ob,
                    )
        for m in range(13, 16):
            c_chunk(m, scalar_copy=True)

    nc.compile()
    return nc


def _numpy_fallback(x, w_q, w_k, w_v, w_out, seg, mask):
    """Exact numpy replica of the reference for non-causal masks."""
    frac = (2.0 * np.arange(Dh // 2, dtype=np.float32)) / Dh
    ts = (MAX_WAVELENGTH ** frac).astype(np.float32)

    def rope(t, pos):
        sinu = pos.astype(np.float32)[:, :, None] / ts  # [B,S,32]
        sn, cs = np.sin(sinu), np.cos(sinu)
        sn, cs = sn[:, :, None, :], cs[:, :, None, :]
        f, s_ = t[..., :32], t[..., 32:]
        return np.concatenate([f * cs - s_ * sn, s_ * cs + f * sn], -1)

    q = np.einsum("bsd,dhk->bshk", x, w_q)
    k = np.einsum("bsd,dhk->bshk", x, w_k)
    v = np.einsum("bsd,dhk->bshk", x, w_v)
    q, k = rope(q, seg), rope(k, seg)
    q = q / np.sqrt(np.float32(Dh))
    attn = np.einsum("bqhd,bkhd->bhqk", q, k)
    attn = np.where(mask, attn, np.finfo(np.float32).min)
    attn = attn - attn.max(-1, keepdims=True)
    e = np.exp(attn)
    attn = e / e.sum(-1, keepdims=True)
    xo = np.einsum("bhqk,bkhd->bqhd", attn, v)
    return np.einsum("bqhd,hdm->bqm", xo, w_out).astype(np.float32)


def _host_inputs(x, w_q, w_k, w_v, w_out, seg):
    frac = (2.0 * np.arange(Dh // 2, dtype=np.float32)) / Dh
    ts = (MAX_WAVELENGTH ** frac).astype(np.float32)
    identb = np.eye(128, dtype=np.float32).astype(BF16)
    keept = np.triu(np.ones((128, 128), dtype=np.float32)).astype(BF16)

    in_maps = []
    for core in range(NCORES):
        b, g = core // 2, core % 2
        hs = slice(g * HL, (g + 1) * HL)
        wq_s = (w_q[:, hs, :] / np.float32(np.sqrt(Dh))).reshape(D, HL * Dh)
        wk_s = w_k[:, hs, :].reshape(D, HL * Dh)
        wv_s = w_v[:, hs, :].reshape(D, HL * Dh)
        wqkv = np.ascontiguousarray(
            np.concatenate([wq_s, wk_s, wv_s], axis=1), dtype=np.float32
        ).astype(BF16)
        woutp = np.stack(
            [
                w_out[g * HL + 2 * p : g * HL + 2 * p + 2].reshape(128, D)
                for p in range(4)
            ]
        ).astype(BF16)
        sinu = seg[b].astype(np.float32)[:, None] / ts  # [S, 32]
        # pre-arrange into partition-major SBUF layouts (contiguous DMAs),
        # si-major so the first S-tile's chunk arrives first
        xt5 = (
            x[b].T.reshape(8, 128, ST, 128).transpose(1, 2, 0, 3)
        )  # [p, si, c, t]
        wq4 = wqkv.astype(np.float32).reshape(8, 128, 3, 512).transpose(
            1, 2, 0, 3
        )  # [p, qkv, c, n]
        wo3 = woutp.transpose(1, 0, 2)  # [p, q, n]
        cos3 = np.cos(sinu).reshape(ST, 128, 32).transpose(1, 0, 2)
        sin3 = np.sin(sinu).reshape(ST, 128, 32).transpose(1, 0, 2)
        in_maps.append(
            {
                "xt": np.ascontiguousarray(xt5).astype(BF16),
                "wqkv": np.ascontiguousarray(wq4).astype(BF16),
                "woutp": np.ascontiguousarray(wo3).astype(BF16),
                "cost": np.ascontiguousarray(cos3).astype(BF16),
                "sint": np.ascontiguousarray(sin3).astype(BF16),
                "identb": identb,
                "keept": keept,
            }
        )
    return in_maps


def _run(in_maps, trace=False):
    from concourse.bass_utils import run_bass_kernel_spmd

    if "nc" not in _CACHE:
        _CACHE["nc"] = _build_bass()
    return run_bass_kernel_spmd(
        _CACHE["nc"], in_maps, core_ids=list(range(NCORES)), trace=trace
    )


def kernel(**inputs):
    x = np.asarray(inputs["inputs"], dtype=np.float32)
    w_q = np.asarray(inputs["w_q"], dtype=np.float32)
    w_k = np.asarray(inputs["w_k"], dtype=np.float32)
    w_v = np.asarray(inputs["w_v"], dtype=np.float32)
    w_out = np.asarray(inputs["w_out"], dtype=np.float32)
    seg = np.asarray(inputs["segment_positions"])
    mask = np.asarray(inputs["mask"])

    causal = np.tril(np.ones((S, S), dtype=bool))
    if not all(np.array_equal(mask[b, 0], causal) for b in range(B)):
        return _numpy_fallback(x, w_q, w_k, w_v, w_out, seg, mask)

    in_maps = _host_inputs(x, w_q, w_k, w_v, w_out, seg)
    res = _run(in_maps)
    outs = [r_["out"] for r_ in res.results]
    result = np.empty((B, S, D), dtype=np.float32)
    for b in range(B):
        result[b] = outs[2 * b] + outs[2 * b + 1]
    return result
